# revision 1
# baseline (speedup 1.0000x reference)
"""Multi-head attention (nn_Attention_987842478290) on 8 TRN2 NeuronCores.

Sharding: batch (2) x head-group (4 groups of 4 heads) = 8 cores; the
host pre-transposes q/k/v per batch and slices Wq/Wk/Wv columns (and Wo
rows) per head group, so every core runs the identical SPMD program on
its shard. Per core:
  - q/k/v projections on PE (bf16 inputs, fp32 PSUM accumulation);
    qh/kh kept TRANSPOSED [head-cols, tokens] so the scores matmul needs
    no on-device transposes. The value matrix gets an extra ones column
    ([vh | 1]) so each AV matmul also produces the softmax row-sums.
  - attention per head pair: S^T tile = [m-tile, headA-n512 | headB-n512]
    via two K=64 matmuls on disjoint PE row groups (they overlap in the
    array); one 1024-wide ACT exp (scale=1/sqrt(64) folded in; scores
    are O(1) so no max-subtraction is needed); AV accumulates
    attn-out^T + row-sums over the 16 m-tiles in PSUM.
  - normalization: row-sums are broadcast across partitions with a K=1
    ones matmul, reciprocal + multiply on DVE, per 512-token chunk.
  - out-projection per token block (interleaved with hp=1's attention),
    bf16-staged stores.
Host: sums the 4 partial outputs per batch and adds bo + bv @ Wo (the
softmax rows sum to 1, so V's bias contributes the constant row bv @ Wo).

Numerics: matmul inputs bf16 or float32r (1 cyc/row); PSUM fp32;
P = exp(S) in bf16; partial outputs bf16, upcast and summed in fp32 on
the host. Measured vs the fp32 reference: rel err ~3e-3.
"""

import numpy as np
import ml_dtypes

import concourse.bass as bass
import concourse.mybir as mybir
import concourse.tile as tile
from concourse.bass_utils import run_bass_kernel_spmd
from concourse.vector_clock import ScopedClock

F32 = mybir.dt.float32
F32R = mybir.dt.float32r
BF16 = mybir.dt.bfloat16
AF = mybir.ActivationFunctionType

B, T, E = 2, 2048, 1024
HEADS, HD = 16, 64
NC_ = 8
GROUPS = 4                  # head-groups (4 heads each)
GC = 256                    # cols per core = 4 heads * 64
P = 128
KC = E // P                 # 8 contraction chunks for projections
NJ = T // 512               # 4 n-chunks of 512
SCALE = 1.0 / np.sqrt(HD)   # 1/8


class SplitDrainTileContext(tile.TileContext):
    """TileContext whose final drain never carries >1 sem wait.

    This walrus build rejects >1 sync-wait per instruction; the stock
    epilogue funnels every outstanding wait onto one SP Drain. Emit the
    extra waits on individual SP nops instead.
    """

    def _drain_and_barrier(self, tick_clock, wait_clock):
        drain_inst = self.nc.sync.drain()
        wait_clock.add_sem_waits(
            drain_inst.ins, ScopedClock({None: tick_clock.global_clock})
        )
        si = drain_inst.ins.sync_info
        waits = list(si.on_wait) if si is not None else []
        if len(waits) > 1:
            import bass_rust

            si.on_wait = waits[:1]
            for w in waits[1:]:
                nop = self.nc.sync.nop(nofuse=True)
                nop.ins.sync_info = bass_rust.SyncInfo(on_wait=[w], on_update=[])

        self.nc.all_engine_barrier()
        assert self.sems is not None
        popped = self.nc._tile_sem_poison_stack.pop()
        assert popped is self._sem_poison
        self.nc.clear_and_free_semaphores(list(self.sems.allocated().values()))
        self.nc.all_engine_barrier()


def _split_multi_waits(nc):
    """Move excess sem waits onto preceding same-engine nops.

    This walrus build accepts at most one sync wait per instruction (two
    for EventSemaphore); Tile's scheduler sometimes attaches more (final
    drain, DMA WAR chains). Each engine executes its block instructions
    in list order, so a nop carrying the extra wait immediately before
    the instruction preserves semantics.
    """
    import bass_rust

    for f in nc.m.functions:
        for bb in f.blocks:
            insts = list(bb.instructions)
            out, changed = [], False
            for inst in insts:
                si = inst.sync_info
                waits = list(si.on_wait) if si is not None else []
                cap = 2 if isinstance(inst, mybir.InstEventSemaphore) else 1
                if len(waits) > cap:
                    changed = True
                    for w in waits[: len(waits) - cap]:
                        nop = mybir.InstNoOp(
                            name=f"I-splitw-{nc.next_id()}",
                            ins=[],
                            outs=[],
                        )
                        nop.engine = inst.engine
                        nop.sync_info = bass_rust.SyncInfo(
                            on_wait=[w], on_update=[]
                        )
                        nc.register_instruction(nop, overwrite=True)
                        out.append(nop)
                    si.on_wait = waits[len(waits) - cap :]
                out.append(inst)
            if changed:
                bb.instructions = out


def build_nc() -> bass.Bass:
    nc = bass.Bass("TRN2", target_bir_lowering=False, debug=False)

    qT = nc.dram_tensor("qT", [E, T], BF16, kind="ExternalInput").ap()
    kT = nc.dram_tensor("kT", [E, T], BF16, kind="ExternalInput").ap()
    vT = nc.dram_tensor("vT", [E, T], BF16, kind="ExternalInput").ap()
    wq = nc.dram_tensor("wq", [E, GC], BF16, kind="ExternalInput").ap()
    wk = nc.dram_tensor("wk", [E, GC], BF16, kind="ExternalInput").ap()
    wv = nc.dram_tensor("wv", [E, GC], BF16, kind="ExternalInput").ap()
    wo = nc.dram_tensor("wo", [GC, E], F32, kind="ExternalInput").ap()
    bq = nc.dram_tensor("bq", [GC], F32, kind="ExternalInput").ap()
    bk = nc.dram_tensor("bk", [GC], F32, kind="ExternalInput").ap()
    out = nc.dram_tensor("out", [T, E], BF16, kind="ExternalOutput").ap()

    with SplitDrainTileContext(nc) as tc:
        _build_body(nc, tc, qT, kT, vT, wq, wk, wv, wo, bq, bk, out)
    _split_multi_waits(nc)
    return nc


def _build_body(nc, tc, qT, kT, vT, wq, wk, wv, wo, bq, bk, out):
    from contextlib import ExitStack

    ctx = ExitStack()
    with ctx:
        cpool = ctx.enter_context(tc.tile_pool(name="consts", bufs=1))
        xpool = ctx.enter_context(tc.tile_pool(name="xstream", bufs=3))
        vpool = ctx.enter_context(tc.tile_pool(name="vblocks", bufs=4))
        ptpool = ctx.enter_context(tc.tile_pool(name="pt", bufs=24))
        rspool = ctx.enter_context(tc.tile_pool(name="rs", bufs=1))
        opool = ctx.enter_context(tc.tile_pool(name="ostage", bufs=2))
        # flat PSUM layout, no stack hand-off: scores 4 banks, AV 2, work 2
        psS = ctx.enter_context(tc.tile_pool(name="psS", bufs=2, space="PSUM"))
        psAV = ctx.enter_context(tc.tile_pool(name="psAV", bufs=2, space="PSUM"))
        psW = ctx.enter_context(tc.tile_pool(name="psW", bufs=2, space="PSUM"))

        # ---- constants ----
        wk_sb = cpool.tile([P, KC, GC], BF16, tag="wk")
        wq_sb = cpool.tile([P, KC, GC], BF16, tag="wq")
        wv_sb = cpool.tile([P, KC, GC], BF16, tag="wv")
        nc.sync.dma_start(wk_sb[:], wk.rearrange("(kc p) c -> p kc c", p=P))
        bq_sb = cpool.tile([P, 2], F32, tag="bq")
        bk_sb = cpool.tile([P, 2], F32, tag="bk")
        nc.sync.dma_start(bq_sb[:], bq.rearrange("(hp p) -> p hp", p=P))
        nc.sync.dma_start(bk_sb[:], bk.rearrange("(hp p) -> p hp", p=P))
        wo_sb = cpool.tile([P, 2, E], F32R, tag="wo")

        # persistent activations
        qhB = [
            [
                cpool.tile(
                    [P, 512], F32R, tag=f"qh{hp}_{j}", name=f"qh{hp}_{j}"
                )
                for j in range(NJ)
            ]
            for hp in range(2)
        ]
        khB = [
            [
                cpool.tile(
                    [P, 512], F32R, tag=f"kh{hp}_{jb}", name=f"kh{hp}_{jb}"
                )
                for jb in range(NJ)
            ]
            for hp in range(2)
        ]
        vh1_tiles = [
            cpool.tile([P, 4, HD + 1], BF16, tag=f"vh1_{i}", name=f"vh1_{i}")
            for i in range(16)
        ]
        attT0 = cpool.tile([P, T], F32R, tag="attT0")     # cols 0..127 (heads 0,1)
        attT1 = cpool.tile([P, T], F32R, tag="attT1")     # cols 128..255 (heads 2,3)
        attTs = [attT0, attT1]
        ones_sb = cpool.tile([P, HD], F32R, tag="ones")
        ones_f32 = cpool.tile([P, HD], F32, tag="ones_f32")
        nc.vector.memset(ones_f32[:], 1.0)
        nc.vector.tensor_copy(out=ones_sb[:], in_=ones_f32[:])

        for i in range(16):
            nc.vector.memset(vh1_tiles[i][:, :, HD : HD + 1], 1.0)

        # ---- phase A: projections, streamed per 512-token block.
        # k first (scores need all of kh), then v (AV needs all of vh),
        # then q (only the j-block of qh gates each attention group).
        def bproj(xdram, w_sb, b_sb, dst_ap, js_range):
            for j in js_range:
                xb = xpool.tile([P, KC, 512], BF16, tag="xb", name=f"xb{j}")
                nc.sync.dma_start(
                    xb[:],
                    xdram[:, j * 512 : (j + 1) * 512].rearrange(
                        "(kc p) t -> p kc t", p=P
                    ),
                )
                for hp in range(2):
                    ps = psW.tile([P, 512], F32, tag="pw", name=f"pj{hp}")
                    for kc in range(KC):
                        nc.tensor.matmul(
                            ps[:],
                            lhsT=w_sb[:, kc, hp * P : (hp + 1) * P],
                            rhs=xb[:, kc, :],
                            start=(kc == 0),
                            stop=(kc == KC - 1),
                        )
                    nc.vector.tensor_scalar_add(
                        dst_ap(hp, j), ps[:], b_sb[:, hp : hp + 1]
                    )

        bproj(kT, wk_sb, bk_sb, lambda hp, j: khB[hp][j][:], range(0, 1))
        nc.sync.dma_start(wq_sb[:], wq.rearrange("(kc p) c -> p kc c", p=P))
        bproj(qT, wq_sb, bq_sb, lambda hp, j: qhB[hp][j][:], range(0, 1))

        def emit_v_block(ib):
            # v: vh natural layout [tokens, cols] (+ ones col for rowsums)
            vb = vpool.tile([P, KC, 512], BF16, tag="vb", name=f"vb{ib}")
            nc.sync.dma_start(
                vb[:],
                vT[:, ib * 512 : (ib + 1) * 512].rearrange(
                    "(kc p) t -> p kc t", p=P
                ),
            )
            for ii in range(4):
                i = 4 * ib + ii
                ps = psW.tile([P, 512], F32, tag="pw")
                for kc in range(KC):
                    nc.tensor.matmul(
                        ps[:, 0:GC],
                        lhsT=vb[:, kc, ii * P : (ii + 1) * P],
                        rhs=wv_sb[:, kc, :],
                        start=(kc == 0),
                        stop=(kc == KC - 1),
                    )
                nc.vector.tensor_copy(
                    out=vh1_tiles[i][:, :, 0:HD],
                    in_=ps[:, 0:GC].rearrange("p (h c) -> p h c", h=4),
                )

        # ---- phase B: attention, head-pair packed ----
        # S tile = [m-tile, (headA n-512 | headB n-512)]: the two score
        # matmuls use disjoint PE row groups (K=64 each) and run
        # concurrently; one 1024-wide exp covers both heads.
        def group_scores_exp(hp, j, collect=None, i_range=range(16)):
            for i in i_range:
                S = psS.tile([P, 1024], F32, tag="S", name="S")
                for hb in range(2):
                    cs = slice(hb * HD, (hb + 1) * HD)
                    nc.tensor.matmul(
                        S[:, hb * 512 : (hb + 1) * 512],
                        lhsT=khB[hp][i // 4][cs, (i % 4) * P : (i % 4 + 1) * P],
                        rhs=qhB[hp][j][cs, :],
                        start=True,
                        stop=True,
                    )
                pt = ptpool.tile([P, 1024], BF16, tag="pt", name=f"pt{i}")
                nc.scalar.activation(pt[:], S[:], AF.Exp, scale=SCALE)
                if collect is not None:
                    collect.append(pt)
                else:
                    yield i, pt

        def group_av_norm(hp, j, rss, pt_iter):
            avA = psAV.tile([HD + 1, 512], F32, tag="av", name="avA")
            avB = psAV.tile([HD + 1, 512], F32, tag="av", name="avB")
            js = slice(j * 512, (j + 1) * 512)
            for i, pt in pt_iter:
                for hb, av in ((0, avA), (1, avB)):
                    nc.tensor.matmul(
                        av[:],
                        lhsT=vh1_tiles[i][:, 2 * hp + hb, :],
                        rhs=pt[:, hb * 512 : (hb + 1) * 512],
                        start=(i == 0),
                        stop=(i == 15),
                    )
            # drain AV psums: attn-out^T rows + rowsum row
            for hb, av in ((0, avA), (1, avB)):
                nc.vector.tensor_copy(
                    out=attTs[hp][hb * HD : (hb + 1) * HD, js],
                    in_=av[0:HD, :],
                )
                nc.vector.tensor_copy(
                    out=rss[hb][0:1, js], in_=av[HD : HD + 1, :]
                )
            # normalize this n-chunk: PE ones-broadcast of the rowsums
            rbA_ps = psW.tile([P, 512], F32, tag="pw", name="rbA_ps")
            rbB_ps = psW.tile([P, 512], F32, tag="pw", name="rbB_ps")
            nc.tensor.matmul(
                rbA_ps[0:HD, :],
                lhsT=ones_sb[0:1, :],
                rhs=rss[0][0:1, js],
                start=True,
                stop=True,
            )
            nc.tensor.matmul(
                rbB_ps[0:HD, :],
                lhsT=ones_sb[0:1, :],
                rhs=rss[1][0:1, js],
                start=True,
                stop=True,
            )
            rbc = rspool.tile([P, 512], F32, tag="rbc")
            nc.vector.reciprocal(rbc[0:HD, :], rbA_ps[0:HD, :])
            nc.vector.reciprocal(rbc[HD:P, :], rbB_ps[0:HD, :])
            nc.vector.tensor_mul(attTs[hp][:, js], attTs[hp][:, js], rbc[:])

            if hp == 1:
                # out-projection for this token block: both attT halves are
                # normalized for n in js once hp=1's chunk is done.
                for e2 in range(2):
                    ost = opool.tile([P, 4, 512], BF16, tag="ost")
                    for ii in range(4):
                        i = j * 4 + ii
                        po = psW.tile([P, 512], F32, tag="pw")
                        for kk in range(2):
                            nc.tensor.matmul(
                                po[:],
                                lhsT=attTs[kk][:, i * P : (i + 1) * P],
                                rhs=wo_sb[:, kk, e2 * 512 : (e2 + 1) * 512],
                                start=(kk == 0),
                                stop=(kk == 1),
                            )
                        nc.vector.tensor_copy(out=ost[:, ii, :], in_=po[:])
                    nc.sync.dma_start(
                        out[
                            j * 512 : (j + 1) * 512, e2 * 512 : (e2 + 1) * 512
                        ].rearrange("(ii p) e -> p ii e", p=P),
                        ost[:],
                    )

        rs0 = (
            rspool.tile([P, T], F32R, tag="rsA", name="rs0A"),
            rspool.tile([P, T], F32R, tag="rsB", name="rs0B"),
        )
        # Head start: the first group's scores/exp run while the v stream
        # and v-proj (which gate AV, not exp) are still in flight; its PT
        # tiles stay buffered until AV catches up.
        # first group's scores/exp interleave with the k-block projections:
        # quarter i=4jb..4jb+3 only needs k-block jb, so the exp stream
        # starts as soon as k0+q0 have landed.
        pts00 = []
        list(group_scores_exp(0, 0, collect=pts00, i_range=range(0, 4)))
        for jb in range(1, NJ):
            bproj(kT, wk_sb, bk_sb, lambda hp, j: khB[hp][j][:], range(jb, jb + 1))
            list(
                group_scores_exp(
                    0, 0, collect=pts00, i_range=range(4 * jb, 4 * jb + 4)
                )
            )

        bproj(qT, wq_sb, bq_sb, lambda hp, j: qhB[hp][j][:], range(1, 2))

        # second head-start: (0,1)'s scores/exp go ahead of v-proj and the
        # AV batches so the exp stream doesn't wait for them on PE.
        pts01 = []
        list(group_scores_exp(0, 1, collect=pts01))

        nc.sync.dma_start(wv_sb[:], wv.rearrange("(kc p) c -> p kc c", p=P))
        for ib in range(4):
            emit_v_block(ib)

        bproj(qT, wq_sb, bq_sb, lambda hp, j: qhB[hp][j][:], range(2, NJ))
        nc.sync.dma_start(
            wo_sb[:], wo.rearrange("(kk p) e -> p kk e", p=P).bitcast(F32R)
        )

        group_av_norm(0, 0, rs0, list(enumerate(pts00)))
        group_av_norm(0, 1, rs0, list(enumerate(pts01)))

        for j in range(2, NJ):
            group_av_norm(0, j, rs0, group_scores_exp(0, j))
        rs1 = (
            rspool.tile([P, T], F32R, tag="rsA", name="rs1A"),
            rspool.tile([P, T], F32R, tag="rsB", name="rs1B"),
        )
        for j in range(NJ):
            group_av_norm(1, j, rs1, group_scores_exp(1, j))


_NC_CACHE: list = []


def kernel(q, k, v, Wq, bq, Wk, bk, Wv, bv, Wo, bo):
    q = np.asarray(q, dtype=np.float32)
    k = np.asarray(k, dtype=np.float32)
    v = np.asarray(v, dtype=np.float32)
    Wq = np.asarray(Wq, dtype=np.float32)
    Wk = np.asarray(Wk, dtype=np.float32)
    Wv = np.asarray(Wv, dtype=np.float32)
    Wo = np.asarray(Wo, dtype=np.float32)
    bq = np.asarray(bq, dtype=np.float32)
    bk = np.asarray(bk, dtype=np.float32)
    bv = np.asarray(bv, dtype=np.float32)
    bo = np.asarray(bo, dtype=np.float32)

    if not _NC_CACHE:
        _NC_CACHE.append(build_nc())
    nc = _NC_CACHE[0]

    bf = ml_dtypes.bfloat16
    qTb = [np.ascontiguousarray(q[b].T).astype(bf) for b in range(B)]
    kTb = [np.ascontiguousarray(k[b].T).astype(bf) for b in range(B)]
    vTb = [np.ascontiguousarray(v[b].T).astype(bf) for b in range(B)]

    in_maps = []
    for c in range(NC_):
        b, g = divmod(c, GROUPS)
        cs = slice(g * GC, (g + 1) * GC)
        in_maps.append(
            {
                "qT": qTb[b],
                "kT": kTb[b],
                "vT": vTb[b],
                "wq": Wq[:, cs].astype(bf),
                "wk": Wk[:, cs].astype(bf),
                "wv": Wv[:, cs].astype(bf),
                "wo": np.ascontiguousarray(Wo[cs, :]),
                "bq": bq[cs],
                "bk": bk[cs],
            }
        )

    kw = {}
    if TRACE:
        kw = dict(trace=True, tmpdir=TRACE_DIR, **TRACE_KW)
    res = run_bass_kernel_spmd(nc, in_maps, core_ids=list(range(NC_)), **kw)
    LAST_RESULT.clear()
    LAST_RESULT.append(res)

    outp = np.zeros((B, T, E), dtype=np.float32)
    for c in range(NC_):
        b = c // GROUPS
        outp[b] += res.results[c]["out"].astype(np.float32)
    # bv's contribution (softmax rows sum to 1): (1 . bv^T) @ Wo, plus bo
    outp += bo + bv @ Wo
    return outp


TRACE = False
TRACE_DIR = None
TRACE_KW: dict = {}
LAST_RESULT: list = []

